# revision 1
# baseline (speedup 1.0000x reference)
"""Multi-head attention (B=4, S=2048, D=1024, H=16) on 8 TRN2 NeuronCores.

Sharding: core c handles batch b = c//2 and head-group g = c%2 (8 heads,
i.e. 512 of the 1024 hidden dims of Wq/Wk/Wv columns, and 512 rows of Wo).
Row-parallel Wo produces per-core partial outputs [S, D]; the host sums
the two partials of each batch (the unshard step of row-parallel layout).

Per-core dataflow (all matmuls on the PE array, float32r / bf16):
  KT[d',s] = (Xk Wk)^T       (bf16, d' on partitions; inputs f32r)
  QT[d',s] = (Xq Wq)^T       (bf16)
  V[s,d']  = Xv Wv           (bf16, with a ones-column appended per head)
  per head h, per q-slab:
    scoresT[k,q] = KT_h^T-slice .T @ QT_h    (PSUM, fp32)
    e = exp(scoresT * scale + maskbias)      (ScalarE, mask folded into
                                              per-partition scale/bias APs)
    av[0:64]  += V_h^T-ish @ e   (attn @ V, transposed; av[64] = denom
                                  via the ones column)
    OT_h = av[0:64] * (1/av[64]) (broadcast via a K=1 matmul)
  out[q,:]  = sum_h OT_h^T-slice.T @ Wo_h    (partial; host adds pairs)

Masking: key positions k >= valid_len get exp() forced to 0 via bias=-1e6.
valid_len==0 gives uniform attention (scale=0, bias=0 -> exp(0)=1), matching
jax.nn.softmax over an all-masked row. The number of key tiles processed
(n_kt) is specialized to max(valid_lens) at kernel-build time.
"""

import math

import numpy as np

B, S, D, H = 4, 2048, 1024, 16
HD = D // H  # 64
NCORES = 8
HPC = 8  # heads per core
DC = 512  # head dims per core
NEG = -1.0e6
P = 128

_PROG_CACHE = {}


def _patch_tile_drain():
    """The walrus build in this container rejects sem waits attached to the
    Tile end-of-kernel Drain ("Too many sync wait commands" / SIGABRT).
    Replace them with standalone EventSemaphore waits, which it accepts."""
    import concourse.tile as tile
    from concourse.vector_clock import ScopedClock

    if getattr(tile.TileContext, "_drain_patched", False):
        return

    def _drain_and_barrier(self, tick_clock, wait_clock):
        nc = self.nc
        drain_inst = nc.sync.drain()
        wait_clock.add_sem_waits(
            drain_inst.ins, ScopedClock({None: tick_clock.global_clock})
        )
        si = drain_inst.ins.sync_info
        waits = list(si.on_wait) if si is not None and si.on_wait else []
        if waits:
            si.on_wait.clear()
            by_id, by_name = {}, {}
            for h in wait_clock.sems.allocated().values():
                by_id[getattr(h, "id", None)] = h
                by_name[getattr(h, "name", None)] = h
            for w in waits:
                h = by_id.get(w.id) or by_name.get(w.ant_name)
                assert h is not None, f"no handle for sem {w.ant_name} ({w.id})"
                nc.sync.wait_ge(h, w.wait_value)
        nc.all_engine_barrier()
        assert self.sems is not None
        popped = nc._tile_sem_poison_stack.pop()
        assert popped is self._sem_poison
        nc.clear_and_free_semaphores(list(self.sems.allocated().values()))
        nc.all_engine_barrier()

    tile.TileContext._drain_and_barrier = _drain_and_barrier
    tile.TileContext._drain_patched = True


def _split_multi_waits(nc, mybir):
    """This container's walrus rejects instructions carrying more than one
    semaphore wait ("Too many sync wait commands"). Hoist excess waits into
    standalone EventSemaphore instructions on the same engine, inserted
    immediately before the instruction — same-engine stream order preserves
    the semantics exactly."""
    n_ev = 0
    for fn in nc.m.functions:
        for bb in fn.blocks:
            insts = bb.instructions
            out = []
            for inst in insts:
                si = inst.sync_info
                waits = list(si.on_wait) if si is not None and si.on_wait else []
                keep = 0 if inst.opcode == "Drain" else 1
                if len(waits) > keep:
                    excess = waits[: len(waits) - keep]
                    kept = waits[len(waits) - keep:]
                    si.on_wait.clear()
                    si.on_wait.extend(kept)
                    for w in excess:
                        ev = mybir.InstEventSemaphore(
                            name=f"{inst.name}-hw{n_ev}",
                            engine=inst.engine,
                        )
                        ev.sync_info = mybir.SyncInfo(on_wait=[w], on_update=[])
                        out.append(ev)
                        n_ev += 1
                out.append(inst)
            if n_ev:
                insts[:] = out
    return n_ev


def _build_program(n_kt: int):
    import concourse.bass as bass
    import concourse.mybir as mybir
    import concourse.tile as tile

    _patch_tile_drain()

    f32 = mybir.dt.float32
    f32r = mybir.dt.float32r
    bf16 = mybir.dt.bfloat16
    AF = mybir.ActivationFunctionType

    nc = bass.Bass()

    xq_d = nc.dram_tensor("xqt", [D, S], f32r, kind="ExternalInput")
    xk_d = nc.dram_tensor("xkt", [D, S], f32r, kind="ExternalInput")
    xv_d = nc.dram_tensor("xvt", [D, S], f32r, kind="ExternalInput")
    wq_d = nc.dram_tensor("wq", [D, DC], f32r, kind="ExternalInput")
    wk_d = nc.dram_tensor("wk", [D, DC], f32r, kind="ExternalInput")
    wv_d = nc.dram_tensor("wv", [D, DC], f32r, kind="ExternalInput")
    wo_d = nc.dram_tensor("wo", [HD, HPC, D], bf16, kind="ExternalInput")
    mb_d = nc.dram_tensor("mb", [P, n_kt], f32, kind="ExternalInput")
    ms_d = nc.dram_tensor("ms", [P, n_kt], f32, kind="ExternalInput")
    out_d = nc.dram_tensor("out", [S, D], f32, kind="ExternalOutput")

    with tile.TileContext(nc) as tc:
        with (
            tc.tile_pool(name="pp", bufs=1) as pp,
            tc.tile_pool(name="wp", bufs=3) as wp,
            tc.tile_pool(name="xtp", bufs=2) as xtp,
            tc.tile_pool(name="expp", bufs=3) as expp,
            tc.tile_pool(name="rcpp", bufs=2) as rcpp,
            tc.tile_pool(name="outp", bufs=2) as outp,
            tc.tile_pool(name="psA", bufs=2, space="PSUM") as psA,
            tc.tile_pool(name="psB", bufs=2, space="PSUM") as psB,
        ):
            # persistent tensors
            KT = pp.tile([P, 4, S], bf16, name="KT")
            QT = pp.tile([P, 4, S], bf16, name="QT")
            V = pp.tile([P, 16, HPC, HD + 1], bf16, name="V")
            OT = [pp.tile([HD, S], bf16, name=f"OT{h}") for h in range(HPC)]
            ones65 = pp.tile([65, P], bf16, name="ones65")
            mb = pp.tile([P, n_kt], f32, name="mb")
            msc = pp.tile([P, n_kt], f32, name="msc")

            nc.any.memset(ones65[:], 1.0)
            nc.sync.dma_start(mb[:], mb_d[:, :])
            nc.sync.dma_start(msc[:], ms_d[:, :])

            wq = wp.tile([P, 8, DC], f32r, name="wq", tag="w")
            wk = wp.tile([P, 8, DC], f32r, name="wk", tag="w")
            wv = wp.tile([P, 8, DC], f32r, name="wv", tag="w")
            nc.sync.dma_start(wq[:], wq_d[:, :].rearrange("(a p) c -> p a c", p=P))
            nc.sync.dma_start(wk[:], wk_d[:, :].rearrange("(a p) c -> p a c", p=P))
            nc.sync.dma_start(wv[:], wv_d[:, :].rearrange("(a p) c -> p a c", p=P))

            # ---- Phase A: KT, QT projections (output transposed: d' on partitions)
            for (x_d, w_sb, dst) in ((xk_d, wk, KT), (xq_d, wq, QT)):
                x_re = x_d[:, :].rearrange("(a p) s -> p a s", p=P)
                for sl in range(8):
                    xs = xtp.tile([P, 8, 256], f32r, name="xs", tag="xt")
                    nc.sync.dma_start(xs[:], x_re[:, :, sl * 256:(sl + 1) * 256])
                    for t in range(4):
                        pj = psA.tile([P, 256], f32, name="pj", tag="A")
                        for a in range(8):
                            nc.tensor.matmul(
                                pj[:],
                                lhsT=w_sb[:, a, t * P:(t + 1) * P],
                                rhs=xs[:, a, :],
                                start=(a == 0),
                                stop=(a == 7),
                            )
                        nc.vector.tensor_copy(
                            out=dst[:, t, sl * 256:(sl + 1) * 256], in_=pj[:]
                        )

            # ---- Phase B: V projection (natural layout: s on partitions)
            xv_re = xv_d[:, :].rearrange("(a p) s -> p a s", p=P)
            for sl in range(8):
                xv = xtp.tile([P, 8, 256], f32r, name="xv", tag="xt")
                nc.sync.dma_start(xv[:], xv_re[:, :, sl * 256:(sl + 1) * 256])
                for sub in range(2):
                    st = sl * 2 + sub
                    pv = psA.tile([P, DC], f32, name="pv", tag="A")
                    for a in range(8):
                        nc.tensor.matmul(
                            pv[:],
                            lhsT=xv[:, a, sub * P:(sub + 1) * P],
                            rhs=wv[:, a, :],
                            start=(a == 0),
                            stop=(a == 7),
                        )
                    nc.any.memset(V[:, st, :, HD:HD + 1], 1.0)
                    nc.vector.tensor_copy(
                        out=V[:, st, :, 0:HD],
                        in_=pv[:].rearrange("p (h c) -> p h c", c=HD),
                    )

            # Wo reuses a weights slot (freed after QT projection consumed wq/wk)
            wo = wp.tile([HD, HPC, D], bf16, name="wo", tag="w")
            nc.sync.dma_start(wo[:], wo_d[:, :, :])

            # ---- Phase C: attention, head pairs interleaved. The even head
            # lives at partitions 0:64 of its KT/QT tile, the odd head at
            # 64:128 — adjacent scores matmuls land in different PE row
            # groups and run concurrently, and the denser PE stream keeps
            # the HAM clock at 8/8.
            for hp in range(HPC // 2):
                t = hp
                for qh in range(2):
                    q0 = qh * 1024
                    avs = [
                        psB.tile([65, 1024], f32, name=f"av{s}", tag="av")
                        for s in range(2)
                    ]
                    for kt in range(n_kt):
                        scs = [
                            psA.tile([P, 1024], f32, name=f"sc{s}", tag="A")
                            for s in range(2)
                        ]
                        for qs in range(2):
                            for s in range(2):
                                pb = s * HD
                                nc.tensor.matmul(
                                    scs[s][:, qs * 512:(qs + 1) * 512],
                                    lhsT=KT[pb:pb + HD, t, kt * P:(kt + 1) * P],
                                    rhs=QT[
                                        pb:pb + HD, t,
                                        q0 + qs * 512:q0 + (qs + 1) * 512,
                                    ],
                                    start=True,
                                    stop=True,
                                )
                        exs = []
                        for s in range(2):
                            ex = expp.tile([P, 1024], bf16, name=f"ex{s}", tag="ex")
                            nc.scalar.activation(
                                ex[:],
                                scs[s][:],
                                AF.Exp,
                                bias=mb[:, kt:kt + 1],
                                scale=msc[:, kt:kt + 1],
                            )
                            exs.append(ex)
                        for qs in range(2):
                            for s in range(2):
                                nc.tensor.matmul(
                                    avs[s][:, qs * 512:(qs + 1) * 512],
                                    lhsT=V[:, kt, 2 * hp + s, :],
                                    rhs=exs[s][:, qs * 512:(qs + 1) * 512],
                                    start=(kt == 0),
                                    stop=(kt == n_kt - 1),
                                )
                    for s in range(2):
                        h = 2 * hp + s
                        # copy to SBUF promptly to release the PSUM bank;
                        # normalization then runs off the PE critical path
                        avb = rcpp.tile([65, 1024], bf16, name="avb", tag="rcp")
                        nc.vector.tensor_copy(out=avb[:], in_=avs[s][:])
                        with nc.allow_low_precision(
                            reason="softmax denominators are O(1e3); bf16 "
                            "reciprocal keeps enough digits for attention"
                        ):
                            nc.vector.reciprocal(avb[64:65, :], avb[64:65, :])
                        bc = psA.tile([P, 1024], f32, name="bc", tag="A")
                        for qs in range(2):
                            nc.tensor.matmul(
                                bc[:, qs * 512:(qs + 1) * 512],
                                lhsT=ones65[64:65, :],
                                rhs=avb[64:65, qs * 512:(qs + 1) * 512],
                                start=True,
                                stop=True,
                            )
                        bcs = expp.tile([P, 1024], bf16, name="bcs", tag="ex")
                        nc.vector.tensor_copy(out=bcs[:], in_=bc[:])
                        nc.vector.tensor_mul(
                            out=OT[h][:, q0:q0 + 1024],
                            in0=avb[0:HD, :],
                            in1=bcs[0:HD, :],
                        )

            # ---- Phase D: output projection (partial over this core's 512 dims)
            for qt in range(16):
                wps = psA.tile([P, 1024], f32, name="wps", tag="A")
                for eh in range(2):
                    for h in range(HPC):
                        nc.tensor.matmul(
                            wps[:, eh * 512:(eh + 1) * 512],
                            lhsT=OT[h][:, qt * P:(qt + 1) * P],
                            rhs=wo[:, h, eh * 512:(eh + 1) * 512],
                            start=(h == 0),
                            stop=(h == HPC - 1),
                        )
                ob = outp.tile([P, 1024], f32, name="ob", tag="ob")
                nc.vector.tensor_copy(out=ob[:], in_=wps[:])
                nc.sync.dma_start(out_d[qt * P:(qt + 1) * P, :], ob[:])

    _split_multi_waits(nc, mybir)
    return nc


def _get_program(n_kt: int):
    if n_kt not in _PROG_CACHE:
        _PROG_CACHE[n_kt] = _build_program(n_kt)
    return _PROG_CACHE[n_kt]


def kernel(**inputs) -> np.ndarray:
    import ml_dtypes
    from concourse.bass_utils import run_bass_kernel_spmd

    q = np.asarray(inputs["queries"], dtype=np.float32)
    k = np.asarray(inputs["keys"], dtype=np.float32)
    v = np.asarray(inputs["values"], dtype=np.float32)
    vl = np.asarray(inputs["valid_lens"]).astype(np.int64)
    Wq = np.asarray(inputs["Wq"], dtype=np.float32)
    Wk = np.asarray(inputs["Wk"], dtype=np.float32)
    Wv = np.asarray(inputs["Wv"], dtype=np.float32)
    Wo = np.asarray(inputs["Wo"], dtype=np.float32)

    if (vl == 0).any():
        n_kt = S // P
    else:
        n_kt = min(S // P, int(math.ceil(vl.max() / P)))
    nc = _get_program(n_kt)

    in_maps = []
    for c in range(NCORES):
        b, g = divmod(c, 2)
        cols = slice(g * DC, (g + 1) * DC)
        wo_sw = np.ascontiguousarray(
            Wo[cols, :].reshape(HPC, HD, D).transpose(1, 0, 2)
        ).astype(ml_dtypes.bfloat16)
        kk = (np.arange(n_kt)[None, :] * P + np.arange(P)[:, None]).astype(np.int64)
        vlb = int(vl[b])
        if vlb == 0:
            m_bias = np.zeros((P, n_kt), np.float32)
            m_scale = np.zeros((P, n_kt), np.float32)
        else:
            m_bias = np.where(kk < vlb, 0.0, NEG).astype(np.float32)
            m_scale = np.full((P, n_kt), 1.0 / math.sqrt(HD), np.float32)
        in_maps.append(
            {
                "xqt": np.ascontiguousarray(q[b].T),
                "xkt": np.ascontiguousarray(k[b].T),
                "xvt": np.ascontiguousarray(v[b].T),
                "wq": np.ascontiguousarray(Wq[:, cols]),
                "wk": np.ascontiguousarray(Wk[:, cols]),
                "wv": np.ascontiguousarray(Wv[:, cols]),
                "wo": wo_sw,
                "mb": m_bias,
                "ms": m_scale,
            }
        )

    globals()["_LAST_IN_MAPS"] = in_maps
    res = run_bass_kernel_spmd(nc, in_maps, list(range(NCORES))).results

    out = np.empty((B, S, D), dtype=np.float32)
    for b in range(B):
        out[b] = res[2 * b]["out"] + res[2 * b + 1]["out"]
    return out



# revision 5
# speedup vs baseline: 1.8572x; 1.8572x over previous
"""Multi-head attention (B=4, S=2048, D=1024, H=16) on 8 TRN2 NeuronCores.

Load-balanced sharding: every core handles heads {2c, 2c+1} of ALL four
batches. Per-batch attention depth kt_b = ceil(valid_len_b / 128) is baked
into the program (identical on every core -> SPMD-safe), so the per-core
work is Sum_b kt_b head-key-tiles regardless of how skewed valid_lens are.
Keys/values are truncated to kt_b*128 rows (masked keys contribute exp=0).

Per-core dataflow (all matmuls bf16 on the PE array):
  QT[b][d',q]   = (Xq[b] Wq_slice)^T      d' = 128 dims of 2 heads
  KT[b][d',k]   = (Xk[b,:Kb] Wk_slice)^T
  V[b][k,s,65]  = Xv[b,:Kb] Wv_slice      (ones column -> softmax denom)
  per (b, q-half, key tile): scores pair (64-dim contraction, heads at
  partition 0:64 / 64:128), exp via ScalarE with mask folded into
  per-partition scale/bias, av accumulation in PSUM.
  normalize: reciprocal_approx_fast on the denominator row, broadcast via
  a K=1 matmul, elementwise mul -> OT; head 1 moved to partitions 64:128
  by an SBUF->SBUF DMA so the O-projection contracts over all 128 dims.
  out_partial[b] = OT[b]^T @ Wo_slice     (bf16 partial, host sums 8)
"""

import math

import numpy as np

B, S, D, H = 4, 2048, 1024, 16
HD = D // H  # 64
NCORES = 8
NEG = -1.0e6
P = 128

_PROG_CACHE = {}


def _patch_tile_drain():
    """The walrus build in this container rejects sem waits attached to the
    Tile end-of-kernel Drain ("Too many sync wait commands" / SIGABRT).
    Replace them with standalone EventSemaphore waits, which it accepts."""
    import concourse.tile as tile
    from concourse.vector_clock import ScopedClock

    if getattr(tile.TileContext, "_drain_patched", False):
        return

    def _drain_and_barrier(self, tick_clock, wait_clock):
        nc = self.nc
        drain_inst = nc.sync.drain()
        wait_clock.add_sem_waits(
            drain_inst.ins, ScopedClock({None: tick_clock.global_clock})
        )
        si = drain_inst.ins.sync_info
        waits = list(si.on_wait) if si is not None and si.on_wait else []
        if waits:
            si.on_wait.clear()
            by_id, by_name = {}, {}
            for h in wait_clock.sems.allocated().values():
                by_id[getattr(h, "id", None)] = h
                by_name[getattr(h, "name", None)] = h
            for w in waits:
                h = by_id.get(w.id) or by_name.get(w.ant_name)
                assert h is not None, f"no handle for sem {w.ant_name} ({w.id})"
                nc.sync.wait_ge(h, w.wait_value)
        nc.all_engine_barrier()
        assert self.sems is not None
        popped = nc._tile_sem_poison_stack.pop()
        assert popped is self._sem_poison
        nc.clear_and_free_semaphores(list(self.sems.allocated().values()))
        nc.all_engine_barrier()

    tile.TileContext._drain_and_barrier = _drain_and_barrier
    tile.TileContext._drain_patched = True


def _split_multi_waits(nc, mybir):
    """This container's walrus rejects instructions carrying more than one
    semaphore wait ("Too many sync wait commands"). Hoist excess waits into
    standalone EventSemaphore instructions on the same engine, inserted
    immediately before the instruction — same-engine stream order preserves
    the semantics exactly."""
    n_ev = 0
    for fn in nc.m.functions:
        for bb in fn.blocks:
            insts = bb.instructions
            out = []
            for inst in insts:
                si = inst.sync_info
                waits = list(si.on_wait) if si is not None and si.on_wait else []
                keep = 0 if inst.opcode == "Drain" else 1
                if len(waits) > keep:
                    excess = waits[: len(waits) - keep]
                    kept = waits[len(waits) - keep:]
                    si.on_wait.clear()
                    si.on_wait.extend(kept)
                    for w in excess:
                        ev = mybir.InstEventSemaphore(
                            name=f"{inst.name}-hw{n_ev}",
                            engine=inst.engine,
                        )
                        ev.sync_info = mybir.SyncInfo(on_wait=[w], on_update=[])
                        out.append(ev)
                        n_ev += 1
                out.append(inst)
            if n_ev:
                insts[:] = out
    return n_ev


def _build_program(kts: tuple):
    import concourse.bass as bass
    import concourse.mybir as mybir
    import concourse.tile as tile

    _patch_tile_drain()

    f32 = mybir.dt.float32
    bf16 = mybir.dt.bfloat16
    AF = mybir.ActivationFunctionType

    KT_total = sum(kts)  # total key tiles across batches
    koff = [0]
    for kt in kts:
        koff.append(koff[-1] + kt)
    SK = KT_total * P  # total truncated key rows

    nc = bass.Bass()

    xq_d = nc.dram_tensor("xqt", [B, D, S], bf16, kind="ExternalInput")
    xk_d = nc.dram_tensor("xkt", [D, SK], bf16, kind="ExternalInput")
    xv_d = nc.dram_tensor("xvt", [D, SK], bf16, kind="ExternalInput")
    wq_d = nc.dram_tensor("wq", [D, P], bf16, kind="ExternalInput")
    wk_d = nc.dram_tensor("wk", [D, P], bf16, kind="ExternalInput")
    wv_d = nc.dram_tensor("wv", [D, P], bf16, kind="ExternalInput")
    wo_d = nc.dram_tensor("wo", [P, D], bf16, kind="ExternalInput")
    mb_d = nc.dram_tensor("mb", [P, KT_total], f32, kind="ExternalInput")
    ms_d = nc.dram_tensor("ms", [P, KT_total], f32, kind="ExternalInput")
    out_d = nc.dram_tensor("out", [B, S, D], bf16, kind="ExternalOutput")

    # process big batches first so their long ScalarE exp streams drain
    # under later batches' PE work
    border = sorted(range(B), key=lambda b: -kts[b])

    with tile.TileContext(nc) as tc:
        with (
            tc.tile_pool(name="pp", bufs=1) as pp,
            tc.tile_pool(name="xp", bufs=2) as xp,
            tc.tile_pool(name="expp", bufs=3) as expp,
            tc.tile_pool(name="dnp", bufs=2) as dnp,
            tc.tile_pool(name="lgp", bufs=2) as lgp,
            tc.tile_pool(name="dnbp", bufs=2) as dnbp,
            tc.tile_pool(name="notp", bufs=2) as notp,
            tc.tile_pool(name="outp", bufs=2) as outp,
            tc.tile_pool(name="psA", bufs=2, space="PSUM") as psA,
            tc.tile_pool(name="psB", bufs=2, space="PSUM") as psB,
        ):
            # persistent tensors
            QT = pp.tile([P, B, S], bf16, name="QT")
            KT = pp.tile([P, SK], bf16, name="KT")
            V = pp.tile([P, KT_total, 2, HD + 1], bf16, name="V")
            OT = pp.tile([P, B, S], bf16, name="OT")
            ones = pp.tile([65, P], bf16, name="ones")
            mb = pp.tile([P, KT_total], f32, name="mb")
            msc = pp.tile([P, KT_total], f32, name="msc")

            nc.any.memset(ones[:], 1.0)
            nc.any.memset(V[:, :, :, HD:HD + 1], 1.0)
            nc.sync.dma_start(mb[:], mb_d[:, :])
            nc.sync.dma_start(msc[:], ms_d[:, :])

            wq = pp.tile([P, 8, P], bf16, name="wq")
            wk = pp.tile([P, 8, P], bf16, name="wk")
            wv = pp.tile([P, 8, P], bf16, name="wv")
            wo = pp.tile([P, D], bf16, name="wo")
            nc.sync.dma_start(wq[:], wq_d[:, :].rearrange("(a p) c -> p a c", p=P))
            nc.sync.dma_start(wk[:], wk_d[:, :].rearrange("(a p) c -> p a c", p=P))
            nc.sync.dma_start(wv[:], wv_d[:, :].rearrange("(a p) c -> p a c", p=P))
            nc.sync.dma_start(wo[:], wo_d[:, :])

            for b in border:
                ktb = kts[b]
                Kb = ktb * P
                kb0 = koff[b] * P  # column offset of batch b in KT / SK axis

                # ---- Q projection: QT[b] [128 d', 2048 q] (transposed out)
                xq_re = xq_d[b, :, :].rearrange("(a p) s -> p a s", p=P)
                for sl in range(2):
                    xt = xp.tile([P, 8, 1024], bf16, name="xt", tag="xt")
                    nc.sync.dma_start(
                        xt[:], xq_re[:, :, sl * 1024:(sl + 1) * 1024]
                    )
                    for sub in range(2):
                        q0 = sl * 1024 + sub * 512
                        ps = psA.tile([P, 512], f32, name="ps", tag="A")
                        for a in range(8):
                            nc.tensor.matmul(
                                ps[:],
                                lhsT=wq[:, a, :],
                                rhs=xt[:, a, sub * 512:(sub + 1) * 512],
                                start=(a == 0),
                                stop=(a == 7),
                            )
                        nc.vector.tensor_copy(
                            out=QT[:, b, q0:q0 + 512], in_=ps[:]
                        )

                # ---- K projection: KT[:, kb0:kb0+Kb] (transposed out)
                xk_re = xk_d[:, :].rearrange("(a p) s -> p a s", p=P)
                for o in range(0, Kb, 1024):
                    w = min(1024, Kb - o)
                    xt = xp.tile([P, 8, w], bf16, name="xtk", tag="xt")
                    nc.sync.dma_start(xt[:], xk_re[:, :, kb0 + o:kb0 + o + w])
                    for so in range(0, w, 512):
                        sw = min(512, w - so)
                        ps = psA.tile([P, sw], f32, name="psk", tag="A")
                        for a in range(8):
                            nc.tensor.matmul(
                                ps[:],
                                lhsT=wk[:, a, :],
                                rhs=xt[:, a, so:so + sw],
                                start=(a == 0),
                                stop=(a == 7),
                            )
                        nc.vector.tensor_copy(
                            out=KT[:, kb0 + o + so:kb0 + o + so + sw], in_=ps[:]
                        )

                # ---- V projection: V[:, gk, s, 0:64] (natural layout)
                xv_re = xv_d[:, :].rearrange("(a p) s -> p a s", p=P)
                for o in range(0, Kb, 1024):
                    w = min(1024, Kb - o)
                    xt = xp.tile([P, 8, w], bf16, name="xtv", tag="xt")
                    nc.sync.dma_start(xt[:], xv_re[:, :, kb0 + o:kb0 + o + w])
                    for loc in range(w // P):
                        gk = koff[b] + (o // P) + loc
                        pv = psA.tile([P, P], f32, name="pv", tag="A")
                        for a in range(8):
                            nc.tensor.matmul(
                                pv[:],
                                lhsT=xt[:, a, loc * P:(loc + 1) * P],
                                rhs=wv[:, a, :],
                                start=(a == 0),
                                stop=(a == 7),
                            )
                        nc.vector.tensor_copy(
                            out=V[:, gk, :, 0:HD],
                            in_=pv[:].rearrange("p (h c) -> p h c", c=HD),
                        )

                # ---- attention per q-half
                for qh in range(2):
                    q0 = qh * 1024
                    avs = [
                        psB.tile([65, 1024], f32, name=f"av{s}", tag="av")
                        for s in range(2)
                    ]
                    for kt in range(ktb):
                        gk = koff[b] + kt
                        scs = [
                            psA.tile([P, 1024], f32, name=f"sc{s}", tag="A")
                            for s in range(2)
                        ]
                        for qs in range(2):
                            for s in range(2):
                                pb = s * HD
                                nc.tensor.matmul(
                                    scs[s][:, qs * 512:(qs + 1) * 512],
                                    lhsT=KT[
                                        pb:pb + HD, kb0 + kt * P:kb0 + (kt + 1) * P
                                    ],
                                    rhs=QT[
                                        pb:pb + HD, b,
                                        q0 + qs * 512:q0 + (qs + 1) * 512,
                                    ],
                                    start=True,
                                    stop=True,
                                )
                        exs = []
                        for s in range(2):
                            ex = expp.tile([P, 1024], bf16, name=f"ex{s}", tag="ex")
                            nc.scalar.activation(
                                ex[:],
                                scs[s][:],
                                AF.Exp,
                                bias=mb[:, gk:gk + 1],
                                scale=msc[:, gk:gk + 1],
                            )
                            exs.append(ex)
                        for qs in range(2):
                            for s in range(2):
                                nc.tensor.matmul(
                                    avs[s][:, qs * 512:(qs + 1) * 512],
                                    lhsT=V[:, gk, s, :],
                                    rhs=exs[s][:, qs * 512:(qs + 1) * 512],
                                    start=(kt == 0),
                                    stop=(kt == ktb - 1),
                                )

                    # normalize: OT[64s:64s+64, b, q0:q0+1024] = av/denom
                    for s in range(2):
                        avb = dnp.tile([65, 1024], f32, name="avb", tag="dn")
                        nc.vector.tensor_copy(
                            out=avb[0:HD, :], in_=avs[s][0:HD, :]
                        )
                        # 1/denom = exp(-ln(denom)) on ScalarE (same table set
                        # as the attention exp; avoids the slow DVE reciprocal)
                        lg = lgp.tile([65, 1024], f32, name="lg", tag="lg")
                        nc.scalar.activation(
                            lg[64:65, :], avs[s][64:65, :], AF.Ln
                        )
                        dnb = dnbp.tile([65, 1024], bf16, name="dnb", tag="dnb")
                        nc.scalar.activation(
                            dnb[64:65, :], lg[64:65, :], AF.Exp, scale=-1.0
                        )
                        bc = psA.tile([P, 1024], f32, name="bc", tag="A")
                        for qs in range(2):
                            nc.tensor.matmul(
                                bc[:, qs * 512:(qs + 1) * 512],
                                lhsT=ones[64:65, :],
                                rhs=dnb[64:65, qs * 512:(qs + 1) * 512],
                                start=True,
                                stop=True,
                            )
                        if s == 0:
                            nc.vector.tensor_mul(
                                out=OT[0:HD, b, q0:q0 + 1024],
                                in0=avb[0:HD, :],
                                in1=bc[0:HD, :],
                            )
                        else:
                            nt = notp.tile([HD, 1024], bf16, name="nt", tag="nt")
                            nc.vector.tensor_mul(
                                out=nt[:], in0=avb[0:HD, :], in1=bc[0:HD, :]
                            )
                            nc.sync.dma_start(
                                OT[HD:P, b, q0:q0 + 1024], nt[:]
                            )

                # ---- O projection: out partial [2048, 1024] bf16
                for ch in range(16):
                    po = psA.tile([P, 1024], f32, name="po", tag="A")
                    for e in range(2):
                        nc.tensor.matmul(
                            po[:, e * 512:(e + 1) * 512],
                            lhsT=OT[:, b, ch * P:(ch + 1) * P],
                            rhs=wo[:, e * 512:(e + 1) * 512],
                            start=True,
                            stop=True,
                        )
                    ob = outp.tile([P, 1024], bf16, name="ob", tag="ob")
                    if ch % 4 == 3:
                        nc.scalar.copy(out=ob[:], in_=po[:])
                    else:
                        nc.vector.tensor_copy(out=ob[:], in_=po[:])
                    nc.sync.dma_start(out_d[b, ch * P:(ch + 1) * P, :], ob[:])

    _split_multi_waits(nc, mybir)
    return nc


def _get_program(kts: tuple):
    if kts not in _PROG_CACHE:
        _PROG_CACHE[kts] = _build_program(kts)
    return _PROG_CACHE[kts]


def kernel(**inputs) -> np.ndarray:
    import ml_dtypes
    from concourse.bass_utils import run_bass_kernel_spmd

    bf = ml_dtypes.bfloat16

    q = np.asarray(inputs["queries"], dtype=np.float32)
    k = np.asarray(inputs["keys"], dtype=np.float32)
    v = np.asarray(inputs["values"], dtype=np.float32)
    vl = np.asarray(inputs["valid_lens"]).astype(np.int64)
    Wq = np.asarray(inputs["Wq"], dtype=np.float32)
    Wk = np.asarray(inputs["Wk"], dtype=np.float32)
    Wv = np.asarray(inputs["Wv"], dtype=np.float32)
    Wo = np.asarray(inputs["Wo"], dtype=np.float32)

    kts = tuple(
        S // P if vl[b] == 0 else min(S // P, int(math.ceil(vl[b] / P)))
        for b in range(B)
    )
    KT_total = sum(kts)
    nc = _get_program(kts)

    # shared (batch-level) arrays — identical on every core
    xqt = np.ascontiguousarray(q.transpose(0, 2, 1)).astype(bf)  # [B, D, S]
    xkt = np.concatenate(
        [k[b, : kts[b] * P].T for b in range(B)], axis=1
    ).astype(bf)  # [D, SK]
    xvt = np.concatenate(
        [v[b, : kts[b] * P].T for b in range(B)], axis=1
    ).astype(bf)

    m_bias = np.empty((P, KT_total), np.float32)
    m_scale = np.empty((P, KT_total), np.float32)
    col = 0
    for b in range(B):
        vlb = int(vl[b])
        for j in range(kts[b]):
            kk = j * P + np.arange(P)
            if vlb == 0:
                m_bias[:, col] = 0.0
                m_scale[:, col] = 0.0
            else:
                m_bias[:, col] = np.where(kk < vlb, 0.0, NEG)
                m_scale[:, col] = 1.0 / math.sqrt(HD)
            col += 1

    in_maps = []
    for c in range(NCORES):
        cols = slice(c * P, (c + 1) * P)  # 2 heads = 128 dims
        in_maps.append(
            {
                "xqt": xqt,
                "xkt": xkt,
                "xvt": xvt,
                "wq": np.ascontiguousarray(Wq[:, cols]).astype(bf),
                "wk": np.ascontiguousarray(Wk[:, cols]).astype(bf),
                "wv": np.ascontiguousarray(Wv[:, cols]).astype(bf),
                "wo": np.ascontiguousarray(Wo[cols, :]).astype(bf),
                "mb": m_bias,
                "ms": m_scale,
            }
        )

    globals()["_LAST_IN_MAPS"] = in_maps
    res = run_bass_kernel_spmd(nc, in_maps, list(range(NCORES))).results

    acc = res[0]["out"].astype(np.float32)
    for c in range(1, NCORES):
        acc += res[c]["out"].astype(np.float32)
    return acc


# revision 9
# speedup vs baseline: 1.9893x; 1.0711x over previous
"""Multi-head attention (B=4, S=2048, D=1024, H=16) on 8 TRN2 NeuronCores.

Load-balanced sharding: every core handles heads {2c, 2c+1} of ALL four
batches. Per-batch attention depth kt_b = ceil(valid_len_b / 128) is baked
into the program (identical on every core -> SPMD-safe), so the per-core
work is Sum_b kt_b head-key-tiles regardless of how skewed valid_lens are.
Keys/values are truncated to kt_b*128 rows (masked keys contribute exp=0).

Per-core dataflow (all matmuls bf16 on the PE array):
  QT[b][d',q]   = (Xq[b] Wq_slice)^T      d' = 128 dims of 2 heads
  KT[b][d',k]   = (Xk[b,:Kb] Wk_slice)^T
  V[b][k,s,65]  = Xv[b,:Kb] Wv_slice      (ones column -> softmax denom)
  per (b, q-half, key tile): scores pair (64-dim contraction, heads at
  partition 0:64 / 64:128), exp via ScalarE with mask folded into
  per-partition scale/bias, av accumulation in PSUM.
  normalize: reciprocal_approx_fast on the denominator row, broadcast via
  a K=1 matmul, elementwise mul -> OT; head 1 moved to partitions 64:128
  by an SBUF->SBUF DMA so the O-projection contracts over all 128 dims.
  out_partial[b] = OT[b]^T @ Wo_slice     (bf16 partial, host sums 8)
"""

import math

import numpy as np

B, S, D, H = 4, 2048, 1024, 16
HD = D // H  # 64
NCORES = 8
NEG = -1.0e6
P = 128

_PROG_CACHE = {}


def _patch_tile_drain():
    """The walrus build in this container rejects sem waits attached to the
    Tile end-of-kernel Drain ("Too many sync wait commands" / SIGABRT).
    Replace them with standalone EventSemaphore waits, which it accepts."""
    import concourse.tile as tile
    from concourse.vector_clock import ScopedClock

    if getattr(tile.TileContext, "_drain_patched", False):
        return

    def _drain_and_barrier(self, tick_clock, wait_clock):
        nc = self.nc
        drain_inst = nc.sync.drain()
        wait_clock.add_sem_waits(
            drain_inst.ins, ScopedClock({None: tick_clock.global_clock})
        )
        si = drain_inst.ins.sync_info
        waits = list(si.on_wait) if si is not None and si.on_wait else []
        if waits:
            si.on_wait.clear()
            by_id, by_name = {}, {}
            for h in wait_clock.sems.allocated().values():
                by_id[getattr(h, "id", None)] = h
                by_name[getattr(h, "name", None)] = h
            for w in waits:
                h = by_id.get(w.id) or by_name.get(w.ant_name)
                assert h is not None, f"no handle for sem {w.ant_name} ({w.id})"
                nc.sync.wait_ge(h, w.wait_value)
        nc.all_engine_barrier()
        assert self.sems is not None
        popped = nc._tile_sem_poison_stack.pop()
        assert popped is self._sem_poison
        nc.clear_and_free_semaphores(list(self.sems.allocated().values()))
        nc.all_engine_barrier()

    tile.TileContext._drain_and_barrier = _drain_and_barrier
    tile.TileContext._drain_patched = True


def _split_multi_waits(nc, mybir):
    """This container's walrus rejects instructions carrying more than one
    semaphore wait ("Too many sync wait commands"). Hoist excess waits into
    standalone EventSemaphore instructions on the same engine, inserted
    immediately before the instruction — same-engine stream order preserves
    the semantics exactly."""
    n_ev = 0
    for fn in nc.m.functions:
        for bb in fn.blocks:
            insts = bb.instructions
            out = []
            for inst in insts:
                si = inst.sync_info
                waits = list(si.on_wait) if si is not None and si.on_wait else []
                keep = 0 if inst.opcode == "Drain" else 1
                if len(waits) > keep:
                    excess = waits[: len(waits) - keep]
                    kept = waits[len(waits) - keep:]
                    si.on_wait.clear()
                    si.on_wait.extend(kept)
                    for w in excess:
                        ev = mybir.InstEventSemaphore(
                            name=f"{inst.name}-hw{n_ev}",
                            engine=inst.engine,
                        )
                        ev.sync_info = mybir.SyncInfo(on_wait=[w], on_update=[])
                        out.append(ev)
                        n_ev += 1
                out.append(inst)
            if n_ev:
                insts[:] = out
    return n_ev


def _build_program(kts: tuple):
    import concourse.bass as bass
    import concourse.mybir as mybir
    import concourse.tile as tile

    _patch_tile_drain()

    f32 = mybir.dt.float32
    bf16 = mybir.dt.bfloat16
    AF = mybir.ActivationFunctionType

    KT_total = sum(kts)  # total key tiles across batches
    koff = [0]
    for kt in kts:
        koff.append(koff[-1] + kt)
    SK = KT_total * P  # total truncated key rows

    nc = bass.Bass()

    xq_d = nc.dram_tensor("xqt", [B, D, S], bf16, kind="ExternalInput")
    xk_d = nc.dram_tensor("xkt", [D, SK], bf16, kind="ExternalInput")
    xv_d = nc.dram_tensor("xvt", [D, SK], bf16, kind="ExternalInput")
    wq_d = nc.dram_tensor("wq", [D, P], bf16, kind="ExternalInput")
    wk_d = nc.dram_tensor("wk", [D, P], bf16, kind="ExternalInput")
    wv_d = nc.dram_tensor("wv", [D, P], bf16, kind="ExternalInput")
    wo_d = nc.dram_tensor("wo", [P, D], bf16, kind="ExternalInput")
    mb_d = nc.dram_tensor("mb", [P, KT_total], f32, kind="ExternalInput")
    ms_d = nc.dram_tensor("ms", [P, KT_total], f32, kind="ExternalInput")
    out_d = nc.dram_tensor("out", [B, S, D], bf16, kind="ExternalOutput")

    # process big batches first so their long ScalarE exp streams drain
    # under later batches' PE work
    border = sorted(range(B), key=lambda b: -kts[b])

    with tile.TileContext(nc) as tc:
        with (
            tc.tile_pool(name="pp", bufs=1) as pp,
            tc.tile_pool(name="xp", bufs=2) as xp,
            tc.tile_pool(name="expp", bufs=3) as expp,
            tc.tile_pool(name="dnp", bufs=2) as dnp,
            tc.tile_pool(name="lgp", bufs=2) as lgp,
            tc.tile_pool(name="dnbp", bufs=2) as dnbp,
            tc.tile_pool(name="notp", bufs=2) as notp,
            tc.tile_pool(name="outp", bufs=2) as outp,
            tc.tile_pool(name="psS", bufs=2, space="PSUM") as psS,
            tc.tile_pool(name="psB", bufs=1, space="PSUM") as psB,
            tc.tile_pool(name="psM", bufs=2, space="PSUM") as psM,
        ):
            # persistent tensors
            QT = pp.tile([P, B, S], bf16, name="QT")
            KT = pp.tile([P, SK], bf16, name="KT")
            V = pp.tile([P, KT_total, 2, HD + 1], bf16, name="V")
            OT = pp.tile([P, B, S], bf16, name="OT")
            ones = pp.tile([65, P], bf16, name="ones")
            mb = pp.tile([P, KT_total], f32, name="mb")
            msc = pp.tile([P, KT_total], f32, name="msc")

            nc.any.memset(ones[:], 1.0)
            nc.any.memset(V[:, :, :, HD:HD + 1], 1.0)
            nc.sync.dma_start(mb[:], mb_d[:, :])
            nc.sync.dma_start(msc[:], ms_d[:, :])

            wq = pp.tile([P, 8, P], bf16, name="wq")
            wk = pp.tile([P, 8, P], bf16, name="wk")
            wv = pp.tile([P, 8, P], bf16, name="wv")
            wo = pp.tile([P, D], bf16, name="wo")
            nc.sync.dma_start(wq[:], wq_d[:, :].rearrange("(a p) c -> p a c", p=P))
            nc.sync.dma_start(wk[:], wk_d[:, :].rearrange("(a p) c -> p a c", p=P))
            nc.sync.dma_start(wv[:], wv_d[:, :].rearrange("(a p) c -> p a c", p=P))
            nc.sync.dma_start(wo[:], wo_d[:, :])

            for b in border:
                ktb = kts[b]
                Kb = ktb * P
                kb0 = koff[b] * P  # column offset of batch b in KT / SK axis

                # ---- Q projection: QT[b] [128 d', 2048 q] (transposed out)
                xq_re = xq_d[b, :, :].rearrange("(a p) s -> p a s", p=P)
                for sl in range(2):
                    xt = xp.tile([P, 8, 1024], bf16, name="xt", tag="xt")
                    nc.sync.dma_start(
                        xt[:], xq_re[:, :, sl * 1024:(sl + 1) * 1024]
                    )
                    for sub in range(2):
                        q0 = sl * 1024 + sub * 512
                        ps = psM.tile([P, 512], f32, name="ps", tag="M")
                        for a in range(8):
                            nc.tensor.matmul(
                                ps[:],
                                lhsT=wq[:, a, :],
                                rhs=xt[:, a, sub * 512:(sub + 1) * 512],
                                start=(a == 0),
                                stop=(a == 7),
                            )
                        nc.vector.tensor_copy(
                            out=QT[:, b, q0:q0 + 512], in_=ps[:]
                        )

                # ---- K projection: KT[:, kb0:kb0+Kb] (transposed out)
                xk_re = xk_d[:, :].rearrange("(a p) s -> p a s", p=P)
                for o in range(0, Kb, 1024):
                    w = min(1024, Kb - o)
                    xt = xp.tile([P, 8, w], bf16, name="xtk", tag="xt")
                    nc.sync.dma_start(xt[:], xk_re[:, :, kb0 + o:kb0 + o + w])
                    for so in range(0, w, 512):
                        sw = min(512, w - so)
                        ps = psM.tile([P, sw], f32, name="psk", tag="M")
                        for a in range(8):
                            nc.tensor.matmul(
                                ps[:],
                                lhsT=wk[:, a, :],
                                rhs=xt[:, a, so:so + sw],
                                start=(a == 0),
                                stop=(a == 7),
                            )
                        nc.vector.tensor_copy(
                            out=KT[:, kb0 + o + so:kb0 + o + so + sw], in_=ps[:]
                        )

                # ---- V projection: V[:, gk, s, 0:64] (natural layout)
                xv_re = xv_d[:, :].rearrange("(a p) s -> p a s", p=P)
                for o in range(0, Kb, 1024):
                    w = min(1024, Kb - o)
                    xt = xp.tile([P, 8, w], bf16, name="xtv", tag="xt")
                    nc.sync.dma_start(xt[:], xv_re[:, :, kb0 + o:kb0 + o + w])
                    for loc in range(w // P):
                        gk = koff[b] + (o // P) + loc
                        pv = psM.tile([P, P], f32, name="pv", tag="M")
                        for a in range(8):
                            nc.tensor.matmul(
                                pv[:],
                                lhsT=xt[:, a, loc * P:(loc + 1) * P],
                                rhs=wv[:, a, :],
                                start=(a == 0),
                                stop=(a == 7),
                            )
                        nc.vector.tensor_copy(
                            out=V[:, gk, :, 0:HD],
                            in_=pv[:].rearrange("p (h c) -> p h c", c=HD),
                        )

                # ---- attention, head-serial so scores stay double-buffered
                # in 4 PSUM banks while misc phases keep their own banks
                for qh in range(2):
                    q0 = qh * 1024
                    for s in range(2):
                        pb = s * HD
                        av = psB.tile([65, 1024], f32, name="av", tag="av")
                        for kt in range(ktb):
                            gk = koff[b] + kt
                            sc = psS.tile([P, 1024], f32, name="sc", tag="S")
                            for qs in range(2):
                                nc.tensor.matmul(
                                    sc[:, qs * 512:(qs + 1) * 512],
                                    lhsT=KT[
                                        pb:pb + HD,
                                        kb0 + kt * P:kb0 + (kt + 1) * P,
                                    ],
                                    rhs=QT[
                                        pb:pb + HD, b,
                                        q0 + qs * 512:q0 + (qs + 1) * 512,
                                    ],
                                    start=True,
                                    stop=True,
                                )
                            ex = expp.tile([P, 1024], bf16, name="ex", tag="ex")
                            nc.scalar.activation(
                                ex[:],
                                sc[:],
                                AF.Exp,
                                bias=mb[:, gk:gk + 1],
                                scale=msc[:, gk:gk + 1],
                            )
                            for qs in range(2):
                                nc.tensor.matmul(
                                    av[:, qs * 512:(qs + 1) * 512],
                                    lhsT=V[:, gk, s, :],
                                    rhs=ex[:, qs * 512:(qs + 1) * 512],
                                    start=(kt == 0),
                                    stop=(kt == ktb - 1),
                                )

                        # normalize: OT[64s:64s+64, b, q0:+1024] = av/denom
                        avb = dnp.tile([65, 1024], f32, name="avb", tag="dn")
                        nc.vector.tensor_copy(out=avb[:], in_=av[:])
                        # 1/denom = exp(-ln(denom)) on ScalarE (same table
                        # set as the attention exp; DVE reciprocal is slow)
                        lg = lgp.tile([65, 1024], f32, name="lg", tag="lg")
                        nc.scalar.activation(
                            lg[64:65, :], avb[64:65, :], AF.Ln
                        )
                        dnb = dnbp.tile([65, 1024], bf16, name="dnb", tag="dnb")
                        nc.scalar.activation(
                            dnb[64:65, :], lg[64:65, :], AF.Exp, scale=-1.0
                        )
                        nt = None
                        if s == 1:
                            nt = notp.tile([HD, 1024], bf16, name="nt", tag="nt")
                        for qs in range(2):
                            bc = psM.tile([P, 512], f32, name="bc", tag="M")
                            nc.tensor.matmul(
                                bc[:],
                                lhsT=ones[64:65, :],
                                rhs=dnb[64:65, qs * 512:(qs + 1) * 512],
                                start=True,
                                stop=True,
                            )
                            qq = q0 + qs * 512
                            if s == 0:
                                nc.vector.tensor_mul(
                                    out=OT[0:HD, b, qq:qq + 512],
                                    in0=avb[0:HD, qs * 512:(qs + 1) * 512],
                                    in1=bc[0:HD, :],
                                )
                            else:
                                nc.vector.tensor_mul(
                                    out=nt[:, qs * 512:(qs + 1) * 512],
                                    in0=avb[0:HD, qs * 512:(qs + 1) * 512],
                                    in1=bc[0:HD, :],
                                )
                                if qs == 1:
                                    nc.sync.dma_start(
                                        OT[HD:P, b, q0:q0 + 1024], nt[:]
                                    )

                # ---- O projection: out partial [2048, 1024] bf16
                for ch in range(16):
                    ob = outp.tile([P, 1024], bf16, name="ob", tag="ob")
                    for e in range(2):
                        po = psM.tile([P, 512], f32, name="po", tag="M")
                        nc.tensor.matmul(
                            po[:],
                            lhsT=OT[:, b, ch * P:(ch + 1) * P],
                            rhs=wo[:, e * 512:(e + 1) * 512],
                            start=True,
                            stop=True,
                        )
                        nc.vector.tensor_copy(
                            out=ob[:, e * 512:(e + 1) * 512], in_=po[:]
                        )
                    nc.sync.dma_start(out_d[b, ch * P:(ch + 1) * P, :], ob[:])

    _split_multi_waits(nc, mybir)
    return nc


def _get_program(kts: tuple):
    if kts not in _PROG_CACHE:
        _PROG_CACHE[kts] = _build_program(kts)
    return _PROG_CACHE[kts]


def kernel(**inputs) -> np.ndarray:
    import ml_dtypes
    from concourse.bass_utils import run_bass_kernel_spmd

    bf = ml_dtypes.bfloat16

    q = np.asarray(inputs["queries"], dtype=np.float32)
    k = np.asarray(inputs["keys"], dtype=np.float32)
    v = np.asarray(inputs["values"], dtype=np.float32)
    vl = np.asarray(inputs["valid_lens"]).astype(np.int64)
    Wq = np.asarray(inputs["Wq"], dtype=np.float32)
    Wk = np.asarray(inputs["Wk"], dtype=np.float32)
    Wv = np.asarray(inputs["Wv"], dtype=np.float32)
    Wo = np.asarray(inputs["Wo"], dtype=np.float32)

    kts = tuple(
        S // P if vl[b] == 0 else min(S // P, int(math.ceil(vl[b] / P)))
        for b in range(B)
    )
    KT_total = sum(kts)
    nc = _get_program(kts)

    # shared (batch-level) arrays — identical on every core
    xqt = np.ascontiguousarray(q.transpose(0, 2, 1)).astype(bf)  # [B, D, S]
    xkt = np.concatenate(
        [k[b, : kts[b] * P].T for b in range(B)], axis=1
    ).astype(bf)  # [D, SK]
    xvt = np.concatenate(
        [v[b, : kts[b] * P].T for b in range(B)], axis=1
    ).astype(bf)

    m_bias = np.empty((P, KT_total), np.float32)
    m_scale = np.empty((P, KT_total), np.float32)
    col = 0
    for b in range(B):
        vlb = int(vl[b])
        for j in range(kts[b]):
            kk = j * P + np.arange(P)
            if vlb == 0:
                m_bias[:, col] = 0.0
                m_scale[:, col] = 0.0
            else:
                m_bias[:, col] = np.where(kk < vlb, 0.0, NEG)
                m_scale[:, col] = 1.0 / math.sqrt(HD)
            col += 1

    in_maps = []
    for c in range(NCORES):
        cols = slice(c * P, (c + 1) * P)  # 2 heads = 128 dims
        in_maps.append(
            {
                "xqt": xqt,
                "xkt": xkt,
                "xvt": xvt,
                "wq": np.ascontiguousarray(Wq[:, cols]).astype(bf),
                "wk": np.ascontiguousarray(Wk[:, cols]).astype(bf),
                "wv": np.ascontiguousarray(Wv[:, cols]).astype(bf),
                "wo": np.ascontiguousarray(Wo[cols, :]).astype(bf),
                "mb": m_bias,
                "ms": m_scale,
            }
        )

    globals()["_LAST_IN_MAPS"] = in_maps
    res = run_bass_kernel_spmd(nc, in_maps, list(range(NCORES))).results

    acc = res[0]["out"].astype(np.float32)
    for c in range(1, NCORES):
        acc += res[c]["out"].astype(np.float32)
    return acc
